# revision 4
# baseline (speedup 1.0000x reference)
"""Trainium2 Bass kernel for NonEquivariantAtomMotifEmbedding.

Computes (pair_embedding [1024,1024,128], dist [1024,1024,14,14]) from
atom positions/masks.  Strategy: shard the first N axis across 8 cores
(128 residue rows each).  All pairwise distances are computed on the PE
via the Gram trick  d2[m,n] = -2*x_m.x_n + (|x_n|^2 + eps) + |x_m|^2,
with the first two terms produced by a single K=4 fp32 matmul (augmented
ones-row carries the |x_n|^2+eps term) and the |x_m|^2 term folded into
the ScalarE Sqrt bias.  fp32 matmuls run at 1/4 rate, recovered via
4-way tile_position row packing.  The RBF->Linear embedding runs with
W as the stationary operand producing E^T tiles; host transposes at
gather time (layout only, no arithmetic).
"""

import sys

sys.path.insert(0, "/opt/trn_rl_repo")

import numpy as np

N, A, NRBF, PS = 1024, 14, 16, 128
NC = 8
NI = N // NC          # 128 rows per core
M_MY = NI * A         # 1792 gram rows per core
NA = N * A            # 14336 gram cols
EPS = 1e-6
EPS_D = 1e-3          # dist-gram eps: guarantees psum>0 under fp32 roundoff
EPS_M = 1e-4          # mean-gram eps
STEP = 22.0 / 16.0
NCH = 448             # dist n-chunk (multiple of 14, fits one PSUM bank)
NNCH = NA // NCH      # 32

# offsets into the fused const tensor (free dim)
O_LT = 0
O_RT = O_LT + M_MY            # 1792
O_BIASD = O_RT + NA           # 16128
O_BIASM = O_BIASD + A         # 16142
O_SCALEV = O_BIASM + 32       # 16174
O_BIASV = O_SCALEV + 1        # 16175
O_WA = O_BIASV + 1            # 16176
O_LTM = O_WA + 128            # 16304
O_RTM = O_LTM + 32 * 128      # 20400
CST_F = O_RTM + N             # 21424

_CACHED_NC = None
_LAST_IN_MAPS = None


def _build_nc():
    import concourse.bass as bass
    import concourse.mybir as mybir
    from concourse import bacc
    from concourse.tile import TileContext
    from contextlib import ExitStack

    f32 = mybir.dt.float32
    AFT = mybir.ActivationFunctionType

    nc = bacc.Bacc()
    CST = nc.dram_tensor("CST", [128, CST_F], f32, kind="ExternalInput")
    EOUT = nc.dram_tensor("EOUT", [NI, 128, N], f32, kind="ExternalOutput")
    DOUT = nc.dram_tensor("DOUT", [M_MY, NA], f32, kind="ExternalOutput")

    with TileContext(nc) as tc, ExitStack() as ctx:
        const = ctx.enter_context(tc.tile_pool(name="const", bufs=1))
        cst = const.tile([128, CST_F], f32)
        nc.sync.dma_start(cst, CST[:, :])
        lt = cst[:, O_LT:O_LT + M_MY]
        rt = cst[:, O_RT:O_RT + NA]
        biasd = cst[:, O_BIASD:O_BIASD + A]
        biasm = cst[:, O_BIASM:O_BIASM + 32]
        scalev = cst[:, O_SCALEV:O_SCALEV + 1]
        biasv = cst[:, O_BIASV:O_BIASV + 1]
        wa = cst[:, O_WA:O_WA + 128]
        ltm = cst[0:4, O_LTM:O_LTM + 32 * 128]
        rtm = cst[0:4, O_RTM:O_RTM + N]

        # ---- Phase E: mean-gram -> dmean -> RBF -> E^T matmul ----
        with tc.tile_pool(name="psm", bufs=2, space="PSUM") as psm_pool, \
             tc.tile_pool(name="pse", bufs=4, space="PSUM") as pse_pool, \
             tc.tile_pool(name="dmp", bufs=2) as dm_pool, \
             tc.tile_pool(name="esb", bufs=4) as esb_pool:
            for g in range(32):
                psm = psm_pool.tile([128, N], f32)
                for jh in range(2):
                    nc.tensor.matmul(
                        psm[:, jh * 512:(jh + 1) * 512],
                        ltm[:, g * 128:(g + 1) * 128],
                        rtm[:, jh * 512:(jh + 1) * 512],
                        start=True, stop=True,
                    )
                # dmean (replicated 32x down partitions), then RBF rows
                dmr = dm_pool.tile([128, N], f32, tag="dmr")
                nc.scalar.activation(dmr, psm, AFT.Sqrt,
                                     bias=biasm[:, g:g + 1], scale=1.0)
                z2 = dm_pool.tile([128, N], f32, tag="z2")
                nc.scalar.activation(z2, dmr, AFT.Square,
                                     bias=biasv, scale=scalev)
                rbf = dm_pool.tile([128, N], f32, tag="rbf")
                nc.scalar.activation(rbf, z2, AFT.Exp, scale=-1.0)
                for s in range(4):
                    i_loc = 4 * g + s
                    for jh in range(2):
                        pse = pse_pool.tile([128, 512], f32)
                        nc.tensor.matmul(
                            pse,
                            wa[32 * s:32 * s + 17, :],
                            rbf[32 * s:32 * s + 17, jh * 512:(jh + 1) * 512],
                            start=True, stop=True,
                            tile_position=(32 * s, 0),
                        )
                        esb = esb_pool.tile([128, 512], f32)
                        nc.vector.tensor_copy(esb, pse)
                        nc.sync.dma_start(
                            EOUT[i_loc, :, jh * 512:(jh + 1) * 512], esb)

        # ---- Phase D: atom-gram -> sqrt -> dist ----
        with tc.tile_pool(name="psd", bufs=6, space="PSUM") as psd_pool, \
             tc.tile_pool(name="dsb", bufs=6) as dsb_pool:
            for gm in range(4):
                nr = 4 if gm < 3 else 2
                for nch in range(NNCH):
                    for r in range(nr):
                        mb = 4 * gm + r
                        psd = psd_pool.tile([128, NCH], f32)
                        nc.tensor.matmul(
                            psd,
                            lt[32 * r:32 * r + 4, mb * 128:(mb + 1) * 128],
                            rt[32 * r:32 * r + 4, nch * NCH:(nch + 1) * NCH],
                            start=True, stop=True,
                            tile_position=(32 * r, 0),
                        )
                        dsb = dsb_pool.tile([128, NCH], f32)
                        nc.scalar.activation(dsb, psd, AFT.Sqrt,
                                             bias=biasd[:, mb:mb + 1], scale=1.0)
                        nc.sync.dma_start(
                            DOUT[mb * 128:(mb + 1) * 128,
                                 nch * NCH:(nch + 1) * NCH], dsb)
    nc.compile()
    return nc


def kernel(**inputs):
    global _CACHED_NC, _LAST_IN_MAPS
    from concourse.bass_utils import run_bass_kernel_spmd

    pos = np.ascontiguousarray(np.asarray(inputs["pos"], np.float32))
    mask = np.ascontiguousarray(np.asarray(inputs["atom_mask"], np.float32))
    W = np.asarray(inputs["W"], np.float32)
    b = np.asarray(inputs["b"], np.float32)

    # host prep: tiny masked-mean / norms + operand layout (0.01% of FLOPs)
    denom = np.maximum(mask.sum(1), 1.0)
    mean = (mask[..., None] * pos).sum(1) / denom[:, None]          # [N,3]
    pf = np.where(mask[..., None] > 0, pos, mean[:, None, :])       # [N,A,3]
    X = pf.reshape(NA, 3)
    nsq = (X * X).sum(1)                                            # [NA]
    nsqm = (mean * mean).sum(1)                                     # [N]
    centers = (np.arange(NRBF, dtype=np.float32) + 0.5) * STEP

    idx32 = np.arange(128) // 32
    in_maps = []
    for c in range(NC):
        i0 = c * NI
        cstv = np.zeros((128, CST_F), np.float32)
        Xm = X[i0 * A:(i0 + NI) * A]                                # [1792,3]
        for r in range(4):
            cstv[32 * r:32 * r + 3, O_LT:O_LT + M_MY] = -2.0 * Xm.T
            cstv[32 * r + 3, O_LT:O_LT + M_MY] = 1.0
            cstv[32 * r:32 * r + 3, O_RT:O_RT + NA] = X.T
            cstv[32 * r + 3, O_RT:O_RT + NA] = nsq + EPS_D
            cstv[32 * r:32 * r + 16, O_WA:O_WA + 128] = W
            cstv[32 * r + 16, O_WA:O_WA + 128] = b
        cstv[:, O_BIASD:O_BIASD + A] = nsq[i0 * A:(i0 + NI) * A].reshape(A, 128).T
        for p in range(128):
            k = p % 32
            if k < NRBF:
                cstv[p, O_SCALEV] = 1.0 / STEP
                cstv[p, O_BIASV] = -centers[k] / STEP
        for g in range(32):
            ii = i0 + 4 * g + idx32
            cstv[0:3, O_LTM + g * 128:O_LTM + (g + 1) * 128] = -2.0 * mean[ii].T
            cstv[3, O_LTM + g * 128:O_LTM + (g + 1) * 128] = 1.0
            cstv[:, O_BIASM + g] = nsqm[ii]
        cstv[0:3, O_RTM:O_RTM + N] = mean.T
        cstv[3, O_RTM:O_RTM + N] = nsqm + EPS_M
        in_maps.append({"CST": cstv})

    if _CACHED_NC is None:
        _CACHED_NC = _build_nc()
    _LAST_IN_MAPS = in_maps
    res = run_bass_kernel_spmd(_CACHED_NC, in_maps, core_ids=list(range(NC)))

    embs, dists = [], []
    for r in res.results:
        embs.append(np.ascontiguousarray(
            r["EOUT"].transpose(0, 2, 1)))                  # [128,1024,128]
        dists.append(np.ascontiguousarray(
            r["DOUT"].reshape(NI, A, N, A).transpose(0, 2, 1, 3)))
    emb = np.concatenate(embs, 0)
    dist = np.concatenate(dists, 0)
    return emb, dist
